# revision 3
# baseline (speedup 1.0000x reference)
"""Trainium2 Bass kernel for CRF score expansion.

Computes crf_scores[b, l, i, j] = emission[b, l, j] + transition[i, j]
for emission [32, 512, 64] f32 and transition [64, 64] f32, output
[32, 512, 64, 64] f32 (256 MB).

Sharding: data-parallel over the batch axis — 8 NeuronCores, 4 batches
(2048 (b,l) rows) per core; transition is replicated. No collectives.

Per-core kernel (v2). Row→partition mapping gives partition p the 16
consecutive rows [16p, 16p+16), so:
  - the whole 512 KB emission shard loads in ONE DMA with one
    contiguous 4 KB descriptor per partition (v1 issued 2048 tiny
    256 B descriptors that competed with the store stream);
  - each output tile u (rows {16p+u}) still stores as 128 contiguous
    16 KB descriptors.
The [T,T] transition is broadcast to all 128 partitions by the Tensor
engine (ones[1,128]^T @ tr_flat[1,4096] into PSUM, 8 bank matmuls,
~2 us) instead of v1's 2 MB stride-0 HBM re-read (~8 us); the DVE add
reads it from PSUM ever after (fp32 tensor_tensor is 1x mode either
way). First and last tiles are split into four [128,1024] sub-tiles so
the first store launches ~3 us earlier and the final add→store serial
tail shrinks. Stores split into 1 MB halves across both HWDGE rings.
The kernel is store-bound: 32 MB/core through 16 SDMA engines at
~27 GB/s each; the DVE add stream (~71 us) hides under it.
"""

import os
from contextlib import ExitStack

import numpy as np

B, L, T = 32, 512, 64
N_CORES = 8
B_PER = B // N_CORES          # 4 batches per core
R = B_PER * L                 # 2048 rows per core
P = 128                       # SBUF partitions
U = R // P                    # 16 rows per partition == tiles
TT = T * T                    # 4096
BANK = 512                    # PSUM bank, fp32 elements
RAMP = 4                      # sub-tiles for first/last tile
SUBW = TT // RAMP             # 1024

_cache = {}

# Set by each kernel() call when tracing is enabled (BASS_KERNEL_TRACE=1):
# the BassKernelResults from run_bass_kernel_spmd, for harnesses that want
# exec_time_ns / trace paths.
last_results = None


def _patch_sem_clear():
    """Replace the raw-ISA EVENT_SEMAPHORE_RANGE_CLEAR (opcode 176) with
    per-sem EventSemaphore writes.

    The walrus build in this container rejects the RANGE_CLEAR encoding
    ("ISA wrong length" in visitInstISA); plain InstEventSemaphore with a
    sem-wr-imm update is lowered by walrus itself and is equivalent for
    the small ranges Tile resets.
    """
    import concourse.bass as bass
    import concourse.mybir as mybir

    if getattr(bass.BassGpSimd, "_sem_clear_patched", False):
        return

    def sem_clear(self, sem):
        nums = list(sem) if isinstance(sem, range) else [sem.num]
        last = None
        for n in nums:
            upd = mybir.SyncUpdate(
                sync_type="semaphore",
                id=n,
                update_mode="sem-wr-imm",
                update_value=0,
                ant_name=f"sem_{n}",
            )
            ins = mybir.InstEventSemaphore(
                name=self.bass.get_next_instruction_name(),
                ins=[],
                outs=[],
                sync_info=mybir.SyncInfo(on_wait=[], on_update=[upd]),
            )
            last = self.add_instruction(ins)
        return last

    for cls in (
        bass.BassGpSimd,
        bass.BassVectorEngine,
        bass.BassScalarEngine,
        bass.BassTensorEngine,
    ):
        cls.sem_clear = sem_clear
    bass.BassGpSimd._sem_clear_patched = True


def _build_bass():
    import concourse.bass as bass
    import concourse.mybir as mybir
    import concourse.tile as tile
    from concourse import bacc

    _patch_sem_clear()

    f32 = mybir.dt.float32
    nc = bacc.Bacc("TRN2", target_bir_lowering=False, debug=False)

    em = nc.dram_tensor("emission", [R, T], f32, kind="ExternalInput")
    tr = nc.dram_tensor("transition", [T, T], f32, kind="ExternalInput")
    out = nc.dram_tensor("out", [R, TT], f32, kind="ExternalOutput")

    # DRAM views for the p ↔ rows [16p, 16p+16) mapping.
    em_v = em[:].rearrange("(p u) j -> p (u j)", p=P)      # [128, 1024]
    out_v = out[:].rearrange("(p u) c -> p (u c)", p=P)    # [128, 65536]

    with ExitStack() as ctx:
        tc = ctx.enter_context(tile.TileContext(nc))
        const_pool = ctx.enter_context(tc.tile_pool(name="const", bufs=1))
        out_pool = ctx.enter_context(tc.tile_pool(name="out", bufs=8))
        ramp_pool = ctx.enter_context(tc.tile_pool(name="ramp", bufs=8))
        psum_pool = ctx.enter_context(
            tc.tile_pool(name="psum", bufs=1, space="PSUM")
        )

        # One contiguous load per input, on separate rings.
        tr_row = const_pool.tile([1, TT], f32)
        nc.sync.dma_start(tr_row[:], tr[:].rearrange("a b -> (a b)").unsqueeze(0))
        em_all = const_pool.tile([P, U * T], f32)
        nc.scalar.dma_start(em_all[:], em_v)

        # trb[p, c] = tr_flat[c] for all p, via PE: ones[1,128]^T @ tr_flat.
        ones_row = const_pool.tile([1, P], f32)
        nc.vector.memset(ones_row[:], 1.0)
        trb = psum_pool.tile([P, TT], f32)
        for k in range(TT // BANK):
            nc.tensor.matmul(
                trb[:, bass.ts(k, BANK)],
                ones_row[:],
                tr_row[:, bass.ts(k, BANK)],
                start=True,
                stop=True,
            )

        rings = [nc.sync, nc.scalar]

        def add_store(u, c0, w, tile_buf):
            """DVE add for columns [c0, c0+w) of tile u, then store the
            two 1 MB-ish halves on both HWDGE rings."""
            ni = w // T
            nc.vector.tensor_add(
                tile_buf[:, :w].rearrange("p (i j) -> p i j", j=T),
                trb[:, c0 : c0 + w].rearrange("p (i j) -> p i j", j=T),
                em_all[:, bass.ts(u, T)].unsqueeze(1).broadcast_to([P, ni, T]),
            )
            h = w // 2
            base = u * TT + c0
            nc.sync.dma_start(out_v[:, base : base + h], tile_buf[:, :h])
            nc.scalar.dma_start(out_v[:, base + h : base + w], tile_buf[:, h:w])

        for u in range(U):
            if u == 0 or u == U - 1:
                for q in range(RAMP):
                    sub = ramp_pool.tile([P, SUBW], f32)
                    add_store(u, q * SUBW, SUBW, sub)
            else:
                o_t = out_pool.tile([P, TT], f32)
                add_store(u, 0, TT, o_t)

    nc.compile()
    return nc


def _get_nc():
    if "nc" not in _cache:
        _cache["nc"] = _build_bass()
    return _cache["nc"]


def kernel(emission: np.ndarray, transition: np.ndarray) -> np.ndarray:
    global last_results
    from concourse.bass_utils import run_bass_kernel_spmd

    nc = _get_nc()

    em = np.ascontiguousarray(emission, dtype=np.float32).reshape(N_CORES, R, T)
    tr = np.ascontiguousarray(transition, dtype=np.float32)
    in_maps = [{"emission": em[i], "transition": tr} for i in range(N_CORES)]

    trace = bool(os.environ.get("BASS_KERNEL_TRACE"))
    res = run_bass_kernel_spmd(
        nc, in_maps, core_ids=list(range(N_CORES)), trace=trace
    )
    if trace:
        last_results = res

    # Undo the p ↔ rows [16p, 16p+16) interleave: device row index is
    # p*U + u for DRAM row 16p + u... the DRAM tensor itself is row-major
    # [R, TT]; the kernel wrote rows in natural order, so no reorder.
    full = np.stack([res.results[i]["out"] for i in range(N_CORES)])
    return full.reshape(B, L, T, T)


# revision 6
# speedup vs baseline: 1.0712x; 1.0712x over previous
"""Trainium2 Bass kernel for CRF score expansion.

Computes crf_scores[b, l, i, j] = emission[b, l, j] + transition[i, j]
for emission [32, 512, 64] f32 and transition [64, 64] f32, output
[32, 512, 64, 64] f32 (256 MB).

Sharding: data-parallel over the batch axis — 8 NeuronCores, 4 batches
(2048 (b,l) rows) per core; transition is replicated. No collectives.

Per-core kernel (v2). Row→partition mapping gives partition p the 16
consecutive rows [16p, 16p+16), so:
  - the whole 512 KB emission shard loads in ONE DMA with one
    contiguous 4 KB descriptor per partition (v1 issued 2048 tiny
    256 B descriptors that competed with the store stream);
  - each output tile u (rows {16p+u}) still stores as 128 contiguous
    16 KB descriptors.
The [T,T] transition is broadcast to all 128 partitions with four
0.5 MB stride-0 DRAM reads (two per HWDGE ring) dispatched before
anything else; the first output tile is split into four [128,1024]
sub-tiles whose adds depend only on the matching trb quarter (Tile
region deps), so the store stream launches ~8 us earlier than one
monolithic broadcast would allow. (A PE ones-matmul broadcast into
PSUM was tried and is slower: fp32 matmul runs 4-pass at ~1 us per
512-col bank and the trailing Tensor DRAIN gates the last reader.)
Stores split into 1 MB halves across both HWDGE rings, which measures
~400 GB/s aggregate vs ~340 for single-ring whole-tile stores. The
kernel is store-bound; the DVE add stream (~71 us) hides under it.
"""

import os
from contextlib import ExitStack

import numpy as np

B, L, T = 32, 512, 64
N_CORES = 8
B_PER = B // N_CORES          # 4 batches per core
R = B_PER * L                 # 2048 rows per core
P = 128                       # SBUF partitions
U = R // P                    # 16 rows per partition == tiles
TT = T * T                    # 4096
BANK = 512                    # PSUM bank, fp32 elements
RAMP = 4                      # sub-tiles for first/last tile
SUBW = TT // RAMP             # 1024

_cache = {}

# Set by each kernel() call when tracing is enabled (BASS_KERNEL_TRACE=1):
# the BassKernelResults from run_bass_kernel_spmd, for harnesses that want
# exec_time_ns / trace paths.
last_results = None


def _patch_sem_clear():
    """Replace the raw-ISA EVENT_SEMAPHORE_RANGE_CLEAR (opcode 176) with
    per-sem EventSemaphore writes.

    The walrus build in this container rejects the RANGE_CLEAR encoding
    ("ISA wrong length" in visitInstISA); plain InstEventSemaphore with a
    sem-wr-imm update is lowered by walrus itself and is equivalent for
    the small ranges Tile resets.
    """
    import concourse.bass as bass
    import concourse.mybir as mybir

    if getattr(bass.BassGpSimd, "_sem_clear_patched", False):
        return

    def sem_clear(self, sem):
        nums = list(sem) if isinstance(sem, range) else [sem.num]
        last = None
        for n in nums:
            upd = mybir.SyncUpdate(
                sync_type="semaphore",
                id=n,
                update_mode="sem-wr-imm",
                update_value=0,
                ant_name=f"sem_{n}",
            )
            ins = mybir.InstEventSemaphore(
                name=self.bass.get_next_instruction_name(),
                ins=[],
                outs=[],
                sync_info=mybir.SyncInfo(on_wait=[], on_update=[upd]),
            )
            last = self.add_instruction(ins)
        return last

    for cls in (
        bass.BassGpSimd,
        bass.BassVectorEngine,
        bass.BassScalarEngine,
        bass.BassTensorEngine,
    ):
        cls.sem_clear = sem_clear
    bass.BassGpSimd._sem_clear_patched = True


def _build_bass():
    import concourse.bass as bass
    import concourse.mybir as mybir
    import concourse.tile as tile
    from concourse import bacc

    _patch_sem_clear()

    f32 = mybir.dt.float32
    nc = bacc.Bacc("TRN2", target_bir_lowering=False, debug=False)

    em = nc.dram_tensor("emission", [R, T], f32, kind="ExternalInput")
    tr = nc.dram_tensor("transition", [T, T], f32, kind="ExternalInput")
    out = nc.dram_tensor("out", [R, TT], f32, kind="ExternalOutput")

    # DRAM views for the p ↔ rows [16p, 16p+16) mapping.
    em_v = em[:].rearrange("(p u) j -> p (u j)", p=P)      # [128, 1024]
    out_v = out[:].rearrange("(p u) c -> p (u c)", p=P)    # [128, 65536]

    with ExitStack() as ctx:
        tc = ctx.enter_context(tile.TileContext(nc))
        const_pool = ctx.enter_context(tc.tile_pool(name="const", bufs=1))
        out_pool = ctx.enter_context(tc.tile_pool(name="out", bufs=8))
        ramp_pool = ctx.enter_context(tc.tile_pool(name="ramp", bufs=4))

        # Broadcast the flattened transition to all 128 partitions with
        # stride-0 DRAM-side APs, in 4 quarters split across both HWDGE
        # rings so the first quarter (which gates the first add) lands
        # early. The emission shard loads first on the scalar ring.
        em_all = const_pool.tile([P, U * T], f32)
        nc.scalar.dma_start(em_all[:], em_v)
        trb = const_pool.tile([P, TT], f32)
        tr_flat = tr[:].rearrange("a b -> (a b)").unsqueeze(0)
        for q in range(RAMP):
            ring = nc.sync if q % 2 == 0 else nc.scalar
            ring.dma_start(
                trb[:, bass.ts(q, SUBW)],
                tr_flat[:, bass.ts(q, SUBW)].broadcast_to([P, SUBW]),
            )

        def add_store(u, c0, w, tile_buf):
            """DVE add for columns [c0, c0+w) of tile u, then store the
            two 1 MB-ish halves on both HWDGE rings."""
            ni = w // T
            nc.vector.tensor_add(
                tile_buf[:, :w].rearrange("p (i j) -> p i j", j=T),
                trb[:, c0 : c0 + w].rearrange("p (i j) -> p i j", j=T),
                em_all[:, bass.ts(u, T)].unsqueeze(1).broadcast_to([P, ni, T]),
            )
            h = w // 2
            base = u * TT + c0
            nc.sync.dma_start(out_v[:, base : base + h], tile_buf[:, :h])
            nc.scalar.dma_start(out_v[:, base + h : base + w], tile_buf[:, h:w])

        for u in range(U):
            if u == 0:
                for q in range(RAMP):
                    sub = ramp_pool.tile([P, SUBW], f32)
                    add_store(u, q * SUBW, SUBW, sub)
            else:
                o_t = out_pool.tile([P, TT], f32)
                add_store(u, 0, TT, o_t)

    nc.compile()
    return nc


def _get_nc():
    if "nc" not in _cache:
        _cache["nc"] = _build_bass()
    return _cache["nc"]


def kernel(emission: np.ndarray, transition: np.ndarray) -> np.ndarray:
    global last_results
    from concourse.bass_utils import run_bass_kernel_spmd

    nc = _get_nc()

    em = np.ascontiguousarray(emission, dtype=np.float32).reshape(N_CORES, R, T)
    tr = np.ascontiguousarray(transition, dtype=np.float32)
    in_maps = [{"emission": em[i], "transition": tr} for i in range(N_CORES)]

    trace = bool(os.environ.get("BASS_KERNEL_TRACE"))
    res = run_bass_kernel_spmd(
        nc, in_maps, core_ids=list(range(N_CORES)), trace=trace
    )
    if trace:
        last_results = res

    # Undo the p ↔ rows [16p, 16p+16) interleave: device row index is
    # p*U + u for DRAM row 16p + u... the DRAM tensor itself is row-major
    # [R, TT]; the kernel wrote rows in natural order, so no reorder.
    full = np.stack([res.results[i]["out"] for i in range(N_CORES)])
    return full.reshape(B, L, T, T)
